# revision 7
# baseline (speedup 1.0000x reference)
"""Fused RoPE attention + LayerNorm, Trainium2, 8 NeuronCores (SPMD).

v2 sharding: 32 (batch, head) units across 8 cores -> each core owns
ONE batch (c//4) and FOUR heads (4*(c%4)..4*(c%4)+3, two row-tile
pairs).  Q/K/V projections are fully local, so there is NO K/V
AllGather at all.  The only collective is a tiny LayerNorm-stats
AllReduce ([1024,2] f32 x2 halves) among each 4-core batch group.

Scores matmuls are packed 2x on the PE via tile_position row tiling:
head-even contracts on array rows 0-63, head-odd on rows 64-127,
running concurrently (validated on HW by probe_rowtile.py).

RoPE is applied as q_rot = u*cos + (perm^T u)*sin with bf16 SBUF
operands so the DVE runs in 2x mode.
"""
import os
import sys
import types
import numpy as np
from contextlib import ExitStack

for _p in ("/opt/trn_rl_repo",):
    if _p not in sys.path:
        sys.path.append(_p)

# NTFF profile hook shim: lets BASS_TRACE=1 work in images whose antenv
# lacks axon_hooks (bass_utils imports it when tracing under axon).
if "antenv.axon_hooks" not in sys.modules:
    _hooks = types.ModuleType("antenv.axon_hooks")
    _HOOK = [None]
    _hooks.set_axon_ntff_profile_hook = lambda h: _HOOK.__setitem__(0, h)
    _hooks.get_axon_ntff_profile_hook = lambda: _HOOK[0]
    sys.modules["antenv.axon_hooks"] = _hooks
    try:
        from trn_agent_boot.trn_boot import _ntff_profile_via_ctypes

        _HOOK[0] = _ntff_profile_via_ctypes("/opt/axon/libaxon_pjrt.so")
    except Exception:
        pass

import concourse.bass as bass  # noqa: E402, F401
import concourse.bacc as bacc  # noqa: E402
import concourse.mybir as mybir  # noqa: E402
import concourse.tile as tile  # noqa: E402
from concourse import bass_utils  # noqa: E402

F32 = mybir.dt.float32
BF16 = mybir.dt.bfloat16
NP_BF16 = np.dtype(mybir.dt.np(BF16))
AF = mybir.ActivationFunctionType
ALU = mybir.AluOpType
AX = mybir.AxisListType

B, S, D, H, DH = 2, 2048, 1024, 16, 64
NC = 8
NP_ = 2              # head pairs per core
NHL = 4              # local heads per core
KT = S // 128        # 16 k-tiles
QC = S // 256        # 8 q-chunks of 256
RC = S // 512        # 4 row chunks of 512 (projection)
DC = D // 128        # 8 contraction chunks
CSL = NHL * 64       # 256 output columns per core
LN_EPS = 1e-5
ROPE_BASE = 10000.0


def _build(flags):
    has_bqk, has_bv, has_gb = flags
    STAGE = int(os.environ.get("KSTAGE", "5"))
    nc = bacc.Bacc("TRN2", target_bir_lowering=False, debug=False,
                   num_devices=NC)

    xqT = nc.dram_tensor("xqT", [D, S], BF16, kind="ExternalInput")
    xvT = nc.dram_tensor("xvT", [D, S], BF16, kind="ExternalInput")
    wq_d = nc.dram_tensor("wq", [D, CSL], BF16, kind="ExternalInput")
    wk_d = nc.dram_tensor("wk", [D, CSL], BF16, kind="ExternalInput")
    wv_d = nc.dram_tensor("wv", [D, CSL], BF16, kind="ExternalInput")
    perm_d = nc.dram_tensor("perm", [128, 128], BF16, kind="ExternalInput")
    ident_d = nc.dram_tensor("ident", [128, 128], BF16, kind="ExternalInput")
    cos_d = nc.dram_tensor("cos", [128, S], BF16, kind="ExternalInput")
    sin_d = nc.dram_tensor("sin", [128, S], BF16, kind="ExternalInput")
    bmask_d = nc.dram_tensor("bmask", [128, NC], F32, kind="ExternalInput")
    if has_bqk:
        cq_d = nc.dram_tensor("cq", [128, NP_ * S], F32, kind="ExternalInput")
        ck_d = nc.dram_tensor("ck", [128, NP_ * S], F32, kind="ExternalInput")
    if has_bv:
        bv_d = nc.dram_tensor("bv", [128, CSL], F32, kind="ExternalInput")
    if has_gb:
        gam_d = nc.dram_tensor("gamma", [128, CSL], F32, kind="ExternalInput")
        bet_d = nc.dram_tensor("beta", [128, CSL], F32, kind="ExternalInput")
    out_d = nc.dram_tensor("out", [S, CSL], F32, kind="ExternalOutput")

    es = ExitStack()
    with es:
        tc = es.enter_context(tile.TileContext(nc))
        dram = es.enter_context(
            tc.tile_pool(name="dram", bufs=1, space="DRAM"))
        constp = es.enter_context(tc.tile_pool(name="const", bufs=1))
        xp = es.enter_context(tc.tile_pool(name="xp", bufs=1))
        wp = es.enter_context(tc.tile_pool(name="wp", bufs=1))
        qkp = es.enter_context(tc.tile_pool(name="qkp", bufs=1))
        vfp = es.enter_context(tc.tile_pool(name="vfp", bufs=1))
        usbp = es.enter_context(tc.tile_pool(name="usbp", bufs=3))
        stagep = es.enter_context(tc.tile_pool(name="stagep", bufs=4))
        ptp = es.enter_context(tc.tile_pool(name="ptp", bufs=24))
        attnp = es.enter_context(tc.tile_pool(name="attnp", bufs=1))
        atsb = es.enter_context(tc.tile_pool(name="atsb", bufs=3))
        epip = es.enter_context(tc.tile_pool(name="epip", bufs=10))
        lnp = es.enter_context(tc.tile_pool(name="lnp", bufs=2))
        outp = es.enter_context(tc.tile_pool(name="outp", bufs=6))
        # PSUM: pj(2 ring x 1 bank) + scp(2 ring x 2 banks) + avp(1) + trp(1)
        pjp = es.enter_context(tc.tile_pool(name="pjp", bufs=2, space="PSUM"))
        scp = es.enter_context(tc.tile_pool(name="scp", bufs=2, space="PSUM"))
        avp = es.enter_context(tc.tile_pool(name="avp", bufs=1, space="PSUM"))
        trp = es.enter_context(tc.tile_pool(name="trp", bufs=1, space="PSUM"))

        st_loc = [dram.tile([128, 2 * QC], F32, tag=f"stl{i}",
                            name=f"stl{i}") for i in range(2)]
        st_sh = [dram.tile([NC * 128, 2 * QC], F32, tag=f"sts{i}",
                           name=f"sts{i}", addr_space="Shared")
                 for i in range(2)]

        # ---- constants + resident inputs ----
        cos_sb = constp.tile([128, S], BF16, tag="cos")
        sin_sb = constp.tile([128, S], BF16, tag="sin")
        perm_sb = constp.tile([128, 128], BF16, tag="perm")
        ident_sb = constp.tile([128, 128], BF16, tag="ident")
        eps_sb = constp.tile([128, 1], F32, tag="eps")
        nc.vector.memset(eps_sb[:], LN_EPS)
        bmask_sb = constp.tile([128, NC], F32, tag="bmask")
        nc.sync.dma_start(bmask_sb[:], bmask_d[:])

        wq_sb = wp.tile([128, DC * CSL], BF16, tag="wq")
        wk_sb = wp.tile([128, DC * CSL], BF16, tag="wk")
        wv_sb = wp.tile([128, DC * CSL], BF16, tag="wv")
        xq_sb = xp.tile([128, DC * S], BF16, tag="xq")
        xv_sb = xp.tile([128, DC * S], BF16, tag="xv")

        nc.sync.dma_start(perm_sb[:], perm_d[:])
        nc.sync.dma_start(cos_sb[:], cos_d[:])
        nc.sync.dma_start(sin_sb[:], sin_d[:])
        nc.sync.dma_start(ident_sb[:], ident_d[:])
        for dc in range(DC):
            nc.sync.dma_start(wk_sb[:, dc * CSL:(dc + 1) * CSL],
                              wk_d[dc * 128:(dc + 1) * 128, :])
        for dc in range(DC):
            nc.sync.dma_start(
                xq_sb[:, dc * S: dc * S + 512],
                xqT[dc * 128:(dc + 1) * 128, 0:512])
        for dc in range(DC):
            nc.sync.dma_start(wq_sb[:, dc * CSL:(dc + 1) * CSL],
                              wq_d[dc * 128:(dc + 1) * 128, :])
        for rc in range(1, RC):
            for dc in range(DC):
                nc.sync.dma_start(
                    xq_sb[:, dc * S + rc * 512: dc * S + (rc + 1) * 512],
                    xqT[dc * 128:(dc + 1) * 128, rc * 512:(rc + 1) * 512])
        for dc in range(DC):
            nc.sync.dma_start(wv_sb[:, dc * CSL:(dc + 1) * CSL],
                              wv_d[dc * 128:(dc + 1) * 128, :])
        for dc in range(DC):
            nc.sync.dma_start(xv_sb[:, dc * S:(dc + 1) * S],
                              xvT[dc * 128:(dc + 1) * 128, :])

        cq_sb = ck_sb = bv_sb = gam_sb = bet_sb = None
        if has_bqk:
            cq_sb = constp.tile([128, NP_ * S], F32, tag="cq")
            ck_sb = constp.tile([128, NP_ * S], F32, tag="ck")
            nc.sync.dma_start(cq_sb[:], cq_d[:])
            nc.sync.dma_start(ck_sb[:], ck_d[:])
        if has_bv:
            bv_sb = constp.tile([128, CSL], F32, tag="bvs")
            nc.sync.dma_start(bv_sb[:], bv_d[:])
        if has_gb:
            gam_sb = constp.tile([128, CSL], F32, tag="gam")
            nc.sync.dma_start(gam_sb[:], gam_d[:])
            bet_sb = constp.tile([128, CSL], F32, tag="bet")
            nc.sync.dma_start(bet_sb[:], bet_d[:])

        # q/k resident, bf16, pair-pair layout; partitions 0-63 = even
        # head dims, 64-127 = odd head dims.
        q_sb = qkp.tile([128, NP_ * S], BF16, tag="q")
        k_sb = qkp.tile([128, NP_ * S], BF16, tag="k")
        # v_full: per (local head h, k-tile kt) a [128, 65] slice at
        # (h*KT + kt)*65; col 64 is the softmax-denominator ones column.
        v_full = vfp.tile([128, NHL * KT * 65], BF16, tag="vf")
        v4 = v_full[:].rearrange("p (h k e) -> p h k e", h=NHL, k=KT)
        nc.vector.memset(v4[:, :, :, 64:65], 1.0)

        # ---- projection emitters ----
        def proj_qk(w_sb, c_sb, dst, p, rc):
            """one (pair, row-chunk) Q-or-K projection + RoPE -> dst."""
            ps_u = pjp.tile([128, 512], F32, tag="pj",
                            name=f"pu{id(w_sb)}_{p}_{rc}")
            for dc in range(DC):
                nc.tensor.matmul(
                    ps_u[:],
                    w_sb[:, dc * CSL + p * 128: dc * CSL + (p + 1) * 128],
                    xq_sb[:, dc * S + rc * 512: dc * S + rc * 512 + 512],
                    start=(dc == 0), stop=(dc == DC - 1))
            u_sb = usbp.tile([128, 512], BF16, tag="usb",
                             name=f"u{id(w_sb)}_{p}_{rc}")
            nc.scalar.copy(u_sb[:], ps_u[:])
            ps_u2 = pjp.tile([128, 512], F32, tag="pj",
                             name=f"pu2{id(w_sb)}_{p}_{rc}")
            nc.tensor.matmul(ps_u2[:], perm_sb[:], u_sb[:],
                             start=True, stop=True)
            u2_sb = usbp.tile([128, 512], BF16, tag="usb",
                              name=f"u2{id(w_sb)}_{p}_{rc}")
            nc.scalar.copy(u2_sb[:], ps_u2[:])
            cs = cos_sb[:, rc * 512:(rc + 1) * 512]
            sn = sin_sb[:, rc * 512:(rc + 1) * 512]
            t1 = stagep.tile([128, 512], BF16, tag="st", name=f"t1_{p}_{rc}")
            nc.vector.tensor_mul(t1[:], u_sb[:], cs)
            t2 = stagep.tile([128, 512], BF16, tag="st", name=f"t2_{p}_{rc}")
            nc.vector.tensor_mul(t2[:], u2_sb[:], sn)
            if c_sb is None:
                nc.vector.tensor_add(dst, t1[:], t2[:])
            else:
                t3 = stagep.tile([128, 512], F32, tag="st3",
                                 name=f"t3_{p}_{rc}")
                nc.vector.tensor_add(t3[:], t1[:], t2[:])
                nc.vector.tensor_add(
                    dst, t3[:], c_sb[:, p * S + rc * 512: p * S + rc * 512 + 512])

        def proj_v(kt, p):
            """one k-tile of V for pair p's two heads -> v_full."""
            ps_v = pjp.tile([128, 512], F32, tag="pj", name=f"pv{kt}_{p}")
            pv = ps_v[:, 0:128]
            for dc in range(DC):
                nc.tensor.matmul(
                    pv,
                    xv_sb[:, dc * S + kt * 128: dc * S + kt * 128 + 128],
                    wv_sb[:, dc * CSL + p * 128: dc * CSL + p * 128 + 128],
                    start=(dc == 0), stop=(dc == DC - 1))
            if has_bv:
                nc.vector.tensor_add(pv, pv, bv_sb[:, p * 128:(p + 1) * 128])
            nc.vector.tensor_copy(
                v4[:, 2 * p:2 * p + 2, kt, 0:64],
                pv.rearrange("p (h e) -> p h e", e=64))

        # ---- attention emitters (q blocks of 512) ----
        def emit_scores_grps(p, qb, g0, g1, pts):
            qs0 = q_sb[0:64, p * S + qb * 512: p * S + qb * 512 + 512]
            qs1 = q_sb[64:128, p * S + qb * 512: p * S + qb * 512 + 512]
            for grp in range(g0, g1):
                ps_s = [scp.tile([128, 1024], F32, tag="sc",
                                 name=f"ss{p}_{qb}_{grp}_{i}")
                        for i in range(2)]
                for jj in range(2):
                    kt = grp * 2 + jj
                    ksl = p * S + kt * 128
                    nc.tensor.matmul(
                        ps_s[0][:, jj * 512:(jj + 1) * 512],
                        k_sb[0:64, ksl: ksl + 128], qs0,
                        start=True, stop=True, tile_position=(0, 0))
                    nc.tensor.matmul(
                        ps_s[1][:, jj * 512:(jj + 1) * 512],
                        k_sb[64:128, ksl: ksl + 128], qs1,
                        start=True, stop=True, tile_position=(64, 0))
                for hh in range(2):
                    pt = ptp.tile([128, 1024], BF16, tag="pt",
                                  name=f"pt{p}_{qb}_{grp}_{hh}")
                    nc.scalar.activation(pt[:], ps_s[hh][:], AF.Exp,
                                         scale=0.125)
                    pts[(grp, hh)] = pt

        def av_alloc(p, qb, hh):
            t = avp.tile([65, 512], F32, tag="av", name=f"aT{p}_{qb}_{hh}")
            return t[:]

        def av_alloc_pj(p, qb, hh):
            # last block: use the (idle-by-now) pj slots to bypass the
            # avp ring-1 serialization with the previous block.
            t = pjp.tile([128, 512], F32, tag="pj", name=f"aTj{p}_{qb}_{hh}")
            return t[0:65, :]

        def av_mms(aT, pts, p, hh, kt0, kt1):
            h = 2 * p + hh
            for kt in range(kt0, kt1):
                nc.tensor.matmul(
                    aT,
                    v_full[:, (h * KT + kt) * 65:
                           (h * KT + kt + 1) * 65],
                    pts[(kt // 2, hh)][:, (kt % 2) * 512:
                                       (kt % 2 + 1) * 512],
                    start=(kt == 0), stop=(kt == KT - 1),
                    skip_group_check=True)

        def av_epilogue(p, qb, aT, hh):
            h = 2 * p + hh
            aT_sb = atsb.tile([65, 512], BF16, tag="ats",
                              name=f"ats{p}_{qb}_{hh}")
            nc.vector.tensor_copy(aT_sb[:], aT)
            tr = trp.tile([128, 264], BF16, tag="tr",
                          name=f"tr{p}_{qb}_{hh}")
            for t in range(4):
                nc.tensor.transpose(
                    tr[:, t * 66: t * 66 + 65],
                    aT_sb[:, t * 128:(t + 1) * 128],
                    ident_sb[0:65, 0:65])
            rec = epip.tile([128, 4], F32, tag="rec",
                            name=f"rec{p}_{qb}_{hh}")
            nc.vector.reciprocal(rec[:], tr[:, 64::66])
            for t in range(4):
                qtg = qb * 4 + t
                nc.vector.tensor_scalar(
                    attn_sb[qtg][:, h * 64:(h + 1) * 64],
                    tr[:, t * 66: t * 66 + 64],
                    rec[:, t: t + 1], None, ALU.mult)

        def emit_stats(qb):
            for t in range(4):
                qtg = qb * 4 + t
                at = attn_sb[qtg]
                stt = epip.tile([128, 2], F32, tag="stt", name=f"stt{qtg}")
                nc.vector.reduce_sum(stt[:, 0:1], at[:], axis=AX.X)
                sq = lnp.tile([128, CSL], F32, tag="sq", name=f"sq{qtg}")
                nc.vector.tensor_mul(sq[:], at[:], at[:])
                nc.vector.reduce_sum(stt[:, 1:2], sq[:], axis=AX.X)
                half, j = qtg // QC, qtg % QC
                nc.sync.dma_start(
                    st_loc[half][:, 2 * j:2 * j + 2], stt[:])

        attn_sb = [attnp.tile([128, CSL], BF16, tag=f"at{t}", name=f"at{t}")
                   for t in range(2 * QC)]

        # ---- emission schedule ----
        # prologue: only K(p0) + Q(p0, rc0) -- the minimum for the first
        # scores block.  All other projections weave into the blocks.
        for rc in range(RC):
            proj_qk(wk_sb, ck_sb, k_sb[:, rc * 512:(rc + 1) * 512], 0, rc)
        proj_qk(wq_sb, cq_sb, q_sb[:, 0:512], 0, 0)

        # ordered so every unit lands before its first consumer:
        # V(p0) before AV(p0,qb0); Q0(rc_i) before S(p0,qb_i);
        # K(p1)+Q1(rc0) before S(p1,qb0); V(p1) before AV(p1,qb0).
        weave = [("v", 0, 0), ("v", 0, 1), ("q", 0, 1),
                 ("v", 0, 2), ("v", 0, 3), ("q", 0, 2),
                 ("q", 0, 3), ("k", 1, 0), ("k", 1, 1),
                 ("k", 1, 2), ("k", 1, 3), ("q", 1, 0),
                 ("v", 1, 0), ("v", 1, 1), ("q", 1, 1),
                 ("v", 1, 2), ("v", 1, 3), ("q", 1, 2),
                 ("q", 1, 3)]

        def emit_weave(n):
            for _ in range(n):
                if not weave:
                    return
                kind, p_, idx = weave.pop(0)
                if kind == "v":
                    for kt in range(idx * 4, idx * 4 + 4):
                        proj_v(kt, p_)
                else:
                    w = wk_sb if kind == "k" else wq_sb
                    c = ck_sb if kind == "k" else cq_sb
                    dstt = k_sb if kind == "k" else q_sb
                    proj_qk(w, c,
                            dstt[:, p_ * S + idx * 512:
                                 p_ * S + idx * 512 + 512], p_, idx)

        blocks = ([(p, qb) for p in range(NP_) for qb in range(4)]
                  if STAGE >= 2 else [])
        pend = None

        def drain_pend(pend):
            # epilogue h0 emitted mid-next-block; h1 at its end
            p_, qb_, pts_, aTs_ = pend
            av_epilogue(p_, qb_, aTs_[1], 1)
            if pend[0] == 1 and STAGE >= 3:
                emit_stats(qb_)
                if qb_ == 1 and STAGE >= 4:
                    nc.gpsimd.collective_compute(
                        "AllGather", ALU.bypass,
                        ins=[st_loc[0][:].opt()],
                        outs=[st_sh[0][:].opt()],
                        replica_groups=[list(range(NC))])

        for i, (p, qb) in enumerate(blocks):
            last = (i == len(blocks) - 1)
            pts = {}
            aTs = {}
            for g in range(8):
                emit_scores_grps(p, qb, g, g + 1, pts)
                emit_weave(1)
                if last:
                    if g == 0:
                        aTs[0] = av_alloc_pj(p, qb, 0)
                    else:
                        av_mms(aTs[0], pts, p, 0, 2 * (g - 1), 2 * g)
                if pend is not None:
                    pp, pq, ppts, paTs = pend
                    if g == 0:
                        paTs[0] = av_alloc(pp, pq, 0)
                    if g < 4:
                        av_mms(paTs[0], ppts, pp, 0, 4 * g, 4 * g + 4)
                    else:
                        if g == 4:
                            av_epilogue(pp, pq, paTs[0], 0)
                            paTs[1] = av_alloc(pp, pq, 1)
                        av_mms(paTs[1], ppts, pp, 1,
                               4 * (g - 4), 4 * (g - 4) + 4)
            if pend is not None:
                drain_pend(pend)
            pend = (p, qb, pts, aTs)
        if STAGE >= 2:
            pp, pq, ppts, paTs = pend
            av_mms(paTs[0], ppts, pp, 0, 14, KT)
            paTs[1] = av_alloc_pj(pp, pq, 1)
            av_mms(paTs[1], ppts, pp, 1, 0, KT)
            av_epilogue(pp, pq, paTs[0], 0)
            av_epilogue(pp, pq, paTs[1], 1)
            emit_weave(99)
        if STAGE >= 3:
            emit_stats(pend[1])
        if STAGE >= 4:
            nc.gpsimd.collective_compute(
                "AllGather", ALU.bypass,
                ins=[st_loc[1][:].opt()], outs=[st_sh[1][:].opt()],
                replica_groups=[list(range(NC))])
        if STAGE < 5:
            for qtg in range(2 * QC):
                at = attn_sb[qtg]
                if STAGE < 2:
                    nc.vector.memset(at[:], 0.0)
                nc.sync.dma_start(out_d[qtg * 128:(qtg + 1) * 128, :], at[:])

        # ---- LayerNorm + store, two halves; half 0 overlaps attention ----
        for half in range(2 if STAGE >= 5 else 0):
            ld = lnp.tile([128, 2 * QC], F32, tag="ld", name=f"ld{half}")
            for r in range(NC):
                ldr = lnp.tile([128, 2 * QC], F32, tag="ldr",
                               name=f"ldr{half}_{r}")
                nc.sync.dma_start(
                    ldr[:], st_sh[half][r * 128:(r + 1) * 128, :])
                if r == 0:
                    nc.vector.tensor_scalar(
                        ld[:], ldr[:], bmask_sb[:, 0:1], None, ALU.mult)
                else:
                    nc.vector.scalar_tensor_tensor(
                        ld[:], ldr[:], bmask_sb[:, r:r + 1], ld[:],
                        ALU.mult, ALU.add)
            mu = epip.tile([128, QC], F32, tag="mu", name=f"mu{half}")
            nc.vector.tensor_scalar_mul(mu[:], ld[:, 0::2], 1.0 / D)
            var = epip.tile([128, QC], F32, tag="va", name=f"va{half}")
            nc.vector.tensor_mul(var[:], mu[:], mu[:])
            nc.vector.tensor_scalar_mul(var[:], var[:], -1.0)
            nc.vector.scalar_tensor_tensor(
                var[:], ld[:, 1::2], 1.0 / D, var[:], ALU.mult, ALU.add)
            std = epip.tile([128, QC], F32, tag="sd", name=f"sd{half}")
            nc.scalar.activation(std[:], var[:], AF.Sqrt, bias=eps_sb[:])
            rstd = epip.tile([128, QC], F32, tag="rs", name=f"rs{half}")
            nc.vector.reciprocal(rstd[:], std[:])
            mrs = epip.tile([128, QC], F32, tag="mr", name=f"mr{half}")
            nc.vector.tensor_mul(mrs[:], mu[:], rstd[:])
            for j in range(QC):
                qtg = half * QC + j
                o_sb = outp.tile([128, CSL], F32, tag="o", name=f"o{qtg}")
                nc.vector.tensor_scalar(
                    o_sb[:], attn_sb[qtg][:], rstd[:, j:j + 1],
                    mrs[:, j:j + 1], ALU.mult, ALU.subtract)
                if has_gb:
                    nc.vector.tensor_mul(o_sb[:], o_sb[:], gam_sb[:])
                    nc.vector.tensor_add(o_sb[:], o_sb[:], bet_sb[:])
                nc.sync.dma_start(
                    out_d[qtg * 128:(qtg + 1) * 128, :], o_sb[:])

    nc.compile()
    return nc


_CACHE: dict = {}
LAST_EXEC_NS = None


def _rope_tables():
    half = DH // 2
    inv_freq = 1.0 / (ROPE_BASE ** (np.arange(half, dtype=np.float32) / half))
    t = np.arange(S, dtype=np.float32)
    freqs = t[:, None] * inv_freq[None, :]
    emb = np.concatenate([freqs, freqs], axis=-1)          # [S, DH]
    return np.cos(emb).astype(np.float32), np.sin(emb).astype(np.float32)


def prep_flags(inputs):
    b_qk = np.asarray(inputs["b_qk"], dtype=np.float32)
    b_v = np.asarray(inputs["b_v"], dtype=np.float32)
    gamma = np.asarray(inputs["ln_gamma"], dtype=np.float32)
    beta = np.asarray(inputs["ln_beta"], dtype=np.float32)
    return (bool(np.any(b_qk)), bool(np.any(b_v)),
            bool(np.any(gamma != 1.0) or np.any(beta != 0.0)))


def _prep_in_maps(inputs, flags):
    x_qk = np.asarray(inputs["x_qk"], dtype=np.float32)
    x_v = np.asarray(inputs["x_v"], dtype=np.float32)
    W_qk = np.asarray(inputs["W_qk"], dtype=np.float32)
    b_qk = np.asarray(inputs["b_qk"], dtype=np.float32)
    W_v = np.asarray(inputs["W_v"], dtype=np.float32)
    b_v = np.asarray(inputs["b_v"], dtype=np.float32)
    gamma = np.asarray(inputs["ln_gamma"], dtype=np.float32)
    beta = np.asarray(inputs["ln_beta"], dtype=np.float32)

    # signed pair-swap: rot2(v)[j] = sum_l Pm[l, j] v[l]
    Pm = np.zeros((128, 128), np.float32)
    for i in range(64):
        Pm[2 * i + 1, 2 * i] = -1.0
        Pm[2 * i, 2 * i + 1] = 1.0
    Pm64 = Pm[:DH, :DH]

    cos_all, sin_all = _rope_tables()
    cos_t = np.tile(cos_all.T, (2, 1))            # [128, S]
    sin_t = np.tile(sin_all.T, (2, 1))
    Wq = W_qk[:, :D]
    Wk = W_qk[:, D:]
    bq = b_qk[:D]
    bk = b_qk[D:]
    bq2 = (bq.reshape(H, DH) @ Pm64).reshape(D)
    bk2 = (bk.reshape(H, DH) @ Pm64).reshape(D)

    perm_np = np.ascontiguousarray(Pm.astype(NP_BF16))
    ident_np = np.ascontiguousarray(np.eye(128, dtype=NP_BF16))
    cos_np = np.ascontiguousarray(cos_t.astype(NP_BF16))
    sin_np = np.ascontiguousarray(sin_t.astype(NP_BF16))

    in_maps = []
    for c in range(NC):
        b = c // 4
        j = c % 4
        cs = slice(CSL * j, CSL * (j + 1))        # this core's D-columns
        bmask = np.zeros((128, NC), np.float32)
        bmask[:, 4 * b: 4 * b + 4] = 1.0
        m = {
            "xqT": np.ascontiguousarray(x_qk[b].T.astype(NP_BF16)),
            "xvT": np.ascontiguousarray(x_v[b].T.astype(NP_BF16)),
            "wq": np.ascontiguousarray(Wq[:, cs].astype(NP_BF16)),
            "wk": np.ascontiguousarray(Wk[:, cs].astype(NP_BF16)),
            "wv": np.ascontiguousarray(W_v[:, cs].astype(NP_BF16)),
            "perm": perm_np, "ident": ident_np,
            "cos": cos_np, "sin": sin_np,
            "bmask": np.ascontiguousarray(bmask),
        }
        if flags[0]:
            # rotated-bias tables per pair: [128, 2*S] f32
            cq = np.empty((128, NP_ * S), np.float32)
            ck = np.empty((128, NP_ * S), np.float32)
            for p in range(NP_):
                ds = slice((4 * j + 2 * p) * 64, (4 * j + 2 * p + 2) * 64)
                cq[:, p * S:(p + 1) * S] = \
                    bq[ds, None] * cos_t + bq2[ds, None] * sin_t
                ck[:, p * S:(p + 1) * S] = \
                    bk[ds, None] * cos_t + bk2[ds, None] * sin_t
            m["cq"] = np.ascontiguousarray(cq)
            m["ck"] = np.ascontiguousarray(ck)
        if flags[1]:
            m["bv"] = np.ascontiguousarray(
                np.broadcast_to(b_v[cs], (128, CSL)).astype(np.float32))
        if flags[2]:
            m["gamma"] = np.ascontiguousarray(
                np.broadcast_to(gamma[cs], (128, CSL)).astype(np.float32))
            m["beta"] = np.ascontiguousarray(
                np.broadcast_to(beta[cs], (128, CSL)).astype(np.float32))
        in_maps.append(m)
    return in_maps


def assemble_output(per_core_outs):
    out = np.empty((B, S, D), np.float32)
    for c in range(NC):
        oc = np.asarray(per_core_outs[c], dtype=np.float32)
        out[c // 4, :, CSL * (c % 4): CSL * (c % 4 + 1)] = oc
    return out


def kernel(**inputs):
    flags = prep_flags(inputs)
    if flags not in _CACHE:
        _CACHE[flags] = _build(flags)
    nc = _CACHE[flags]
    in_maps = _prep_in_maps(inputs, flags)
    res = bass_utils.run_bass_kernel_spmd(
        nc, in_maps, core_ids=list(range(NC)))
    global LAST_EXEC_NS
    LAST_EXEC_NS = res.exec_time_ns
    return assemble_output([res.results[c]["out"] for c in range(NC)])
